# revision 20
# baseline (speedup 1.0000x reference)
"""Trainium2 Bass kernel for AtlasMemoryPoly (dense_mlp).

Reference (DIM=256, HIDDEN=1024, POLY=33152, x:(2,1024,256)):
    x_poly = [x, x_i*x_j for i<=j]                  # (T=2048, P=33152)
    gate   = silu(x_poly @ w2.T)                    # (T, H)
    value  = x_poly @ w3.T                          # (T, H)
    out    = x + (gate*value) @ w1.T                # (T, D)

Sharding: 8 cores = 4 h-groups (256 each) x 2 t-groups (1024 each).
Each core computes its (t_local, h_local) block and a partial output
(1024, 256); the host sums the 4 h-group partials per t-group, adds x.

Poly features: the 33152 poly axis is PERMUTED host-side (same
permutation applied to w2/w3 rows) into PAIRS of 128-feature tiles.
Pair q covers tiles (2q, 2q+1), generated as ONE elementwise multiply
of stacked row-window tiles (partition p, j in {0,1}):
    q0: [X0;X1] copy              (linear features)
    q1: [X0;X1] * [X0;X1]         (squares)
    q2: [X0*X1 ; ZERO-PAD]        (antipodal + pad to 260 tiles)
    q(2+d), d=1..127: [X0;X1] * xt2[d:256+d]
          j=0: X0*xT[d:d+128]      -> pairs (i, i+d)
          j=1: X1*xT[128+d:256+d]  -> pairs (128+i, (128+i+d)%256)
xt2 is xT doubled (512 rows) so every window is one contiguous DMA.

Precision split: pairs q < QBF stay bf16 (DVE 2x multiply, plain bf16
matmuls); pairs q >= QBF are written as fp8e4 (DVE 1x) and consumed by
DoubleRow matmuls (K=256, 2 MACs/PE/cycle). All w2/w3 weights are
host-scaled by 256 (fp8e4 min-normal headroom); the epilogue applies
1/256 via ACT scale. This balances DVE / PE / DMA.
"""

import sys

sys.path.insert(0, "/opt/trn_rl_repo")

import numpy as np
import ml_dtypes

DIM = 256
HIDDEN = 1024
T = 2048
POLY = DIM + DIM * (DIM + 1) // 2  # 33152
NPAIR = 130  # 260 tiles of 128 (one zero pad tile)
NCORES = 8
NHG = 2
NTG = 4
HLOC = HIDDEN // NHG  # 512
TLOC = T // NTG  # 512
NHC = HLOC // 128  # 4 h-chunks
W_SCALE = 256.0

BF16 = ml_dtypes.bfloat16
FP8 = ml_dtypes.float8_e4m3fn


def build_perm():
    """tile-row index (260*128) -> old poly row, or -1 for the pad tile.

    Tile order: [lin0, lin1, sq0, sq1, anti, PAD, then (A_d, B_d) for
    d=1..127] where A_d rows i are pairs (i, i+d) and B_d rows i are
    pairs (128+i, (128+i+d) % 256).
    """
    i = np.arange(128)

    def pairs_to_old(a, b):
        lo = np.minimum(a, b)
        hi = np.maximum(a, b)
        return DIM + lo * DIM - lo * (lo - 1) // 2 + (hi - lo)

    chunks = [
        np.arange(0, 128),                  # lin0
        np.arange(128, 256),                # lin1
        pairs_to_old(i, i),                 # sq0
        pairs_to_old(128 + i, 128 + i),     # sq1
        pairs_to_old(i, 128 + i),           # anti
        np.full(128, -1, dtype=np.int64),   # PAD
    ]
    for d in range(1, 128):
        chunks.append(pairs_to_old(i, i + d))
        j = (128 + i + d) % 256
        chunks.append(pairs_to_old(128 + i, j))
    return np.concatenate(chunks)


_NC_CACHE = None


def _build_nc():
    from concourse import bacc, tile, mybir
    from concourse.mybir import ActivationFunctionType as AF

    nc = bacc.Bacc()
    bf = mybir.dt.bfloat16
    f8 = mybir.dt.float8e4
    f32 = mybir.dt.float32
    DR = mybir.MatmulPerfMode.DoubleRow

    xt_d = nc.dram_tensor("xt", (2 * DIM, TLOC), bf, kind="ExternalInput")
    wf8_d = nc.dram_tensor("wf8", (2, NPAIR, 128, 2, HLOC), f8, kind="ExternalInput")
    w1_d = nc.dram_tensor("w1s", (HLOC, DIM), bf, kind="ExternalInput")
    out_d = nc.dram_tensor("out", (TLOC, DIM), f32, kind="ExternalOutput")

    with tile.TileContext(nc) as tc:
        with (
            tc.tile_pool(name="xpool", bufs=1) as xpool,
            tc.tile_pool(name="shift", bufs=14) as shift,
            tc.tile_pool(name="poly", bufs=14) as poly,
            tc.tile_pool(name="wts", bufs=20) as wts,
            tc.tile_pool(name="epi", bufs=1) as epi,
            tc.tile_pool(name="ostage", bufs=4) as ostage,
            tc.tile_pool(name="psum", bufs=1, space="PSUM") as psum,
        ):
            # XX = [X0; X1] stacked pair tile (128, 2*TLOC): j-major halves
            XX = xpool.tile([128, 2 * TLOC], bf, tag="XX")
            nc.sync.dma_start(
                XX.rearrange("p (j t) -> p j t", j=2),
                xt_d[0:256, :].rearrange("(j p) t -> p j t", p=128),
            )

            acc = {}
            for w in (0, 1):
                for hc in range(NHC):
                    acc[(w, hc)] = psum.tile(
                        [128, TLOC], f32, tag=f"acc{w}{hc}", name=f"acc{w}{hc}"
                    )

            def pair_tile(q, dtype):
                """(128, 2*TLOC) tile holding poly tiles (2q, 2q+1)."""
                pt = poly.tile([128, 2 * TLOC], dtype, tag="poly", name=f"pt{q}")
                if q == 0:
                    nc.vector.tensor_copy(pt[:], XX[:])
                elif q == 1:
                    nc.vector.tensor_mul(pt[:], XX[:], XX[:])
                elif q == 2:
                    nc.vector.tensor_mul(
                        pt[:, 0:TLOC], XX[:, 0:TLOC], XX[:, TLOC : 2 * TLOC]
                    )
                    nc.vector.memset(pt[:, TLOC : 2 * TLOC], 0.0)
                else:
                    d = q - 2
                    sw = shift.tile([128, 2 * TLOC], bf, tag="sd", name=f"sw{q}")
                    nc.sync.dma_start(
                        sw.rearrange("p (j t) -> p j t", j=2),
                        xt_d[d : d + 256, :].rearrange("(j p) t -> p j t", p=128),
                    )
                    nc.vector.tensor_mul(pt[:], XX[:], sw[:])
                return pt

            for q in range(NPAIR):
                pt = pair_tile(q, f8)
                pt3 = pt.rearrange("p (j t) -> p j t", j=2)
                st = q == 0
                sp = q == NPAIR - 1
                wtiles = []
                for w in (0, 1):
                    wt = wts.tile([128, 2, HLOC], f8, tag="wf8", name=f"wf{w}_{q}")
                    nc.sync.dma_start(wt[:], wf8_d[w, q])
                    wtiles.append(wt)
                for w in (0, 1):
                    wt = wtiles[w]
                    for hc in range(NHC):
                        hsl = slice(hc * 128, (hc + 1) * 128)
                        nc.tensor.matmul(
                            acc[(w, hc)][:],
                            wt[:, :, hsl],
                            pt3[:],
                            start=st,
                            stop=sp,
                            perf_mode=DR,
                        )

            # epilogue per h-chunk: gated = silu(gate/256) * (value/256), bf16
            gated = {}
            for hc in range(NHC):
                sil = epi.tile([128, TLOC], bf, tag=f"sil{hc}", name=f"sil{hc}")
                vv = epi.tile([128, TLOC], bf, tag=f"vv{hc}", name=f"vv{hc}")
                g = epi.tile([128, TLOC], bf, tag=f"gated{hc}", name=f"g{hc}")
                nc.scalar.activation(
                    sil[:], acc[(0, hc)][:], AF.Silu, scale=1.0 / W_SCALE
                )
                nc.scalar.activation(
                    vv[:], acc[(1, hc)][:], AF.Copy, scale=1.0 / W_SCALE
                )
                nc.vector.tensor_mul(g[:], sil[:], vv[:])
                gated[hc] = g

            w1t = {}
            for hc in range(NHC):
                wt1 = xpool.tile([128, DIM], bf, tag=f"w1_{hc}", name=f"w1_{hc}")
                nc.sync.dma_start(wt1[:], w1_d[hc * 128 : (hc + 1) * 128, :])
                w1t[hc] = wt1

            for tc_i in range(TLOC // 128):
                ops = psum.tile(
                    [128, DIM],
                    f32,
                    tag=f"acc{tc_i % 2}{(tc_i // 2) % 2}",
                    name=f"ops{tc_i}",
                )
                tsl = slice(tc_i * 128, (tc_i + 1) * 128)
                for hc in range(NHC):
                    nc.tensor.matmul(
                        ops[:],
                        gated[hc][:, tsl],
                        w1t[hc][:],
                        start=hc == 0,
                        stop=hc == NHC - 1,
                    )
                ost = ostage.tile([128, DIM], f32, tag="ost", name=f"ost{tc_i}")
                nc.scalar.copy(ost[:], ops[:])
                nc.sync.dma_start(out_d[tsl, :], ost[:])

    nc.finalize()
    return nc


def _get_nc():
    global _NC_CACHE
    if _NC_CACHE is None:
        _NC_CACHE = _build_nc()
    return _NC_CACHE


def prepare_inputs(x, w1, w2, w3):
    """Host-side shard prep. Returns in_maps for the 8 cores."""
    perm = build_perm()  # (260*128,) with -1 for pad rows
    xt1 = np.ascontiguousarray(x.reshape(T, DIM).T).astype(BF16)  # (256, 2048)
    xt2 = np.concatenate([xt1, xt1], axis=0)  # (512, 2048)

    def to_pairs(w):  # (HIDDEN, POLY) -> (NPAIR, 128, 2, HIDDEN) f32 scaled
        wt = w.T * W_SCALE  # (POLY, HIDDEN)
        wt = np.concatenate([wt, np.zeros((1, HIDDEN), wt.dtype)], axis=0)
        g = wt[perm]  # perm -1 -> last (zero) row
        # row layout: pair q, tile j, partition k  ->  row (2q+j)*128+k
        return g.reshape(NPAIR, 2, 128, HIDDEN).transpose(0, 2, 1, 3)

    w2p = to_pairs(w2)
    w3p = to_pairs(w3)
    w1t = np.ascontiguousarray(w1.T).astype(BF16)  # (1024, 256)

    in_maps = []
    for c in range(NCORES):
        tg, hg = divmod(c, NHG)
        tsl = slice(tg * TLOC, (tg + 1) * TLOC)
        hsl = slice(hg * HLOC, (hg + 1) * HLOC)
        wf8 = np.stack([w2p[:, :, :, hsl], w3p[:, :, :, hsl]]).astype(FP8)
        in_maps.append(
            {
                "xt": np.ascontiguousarray(xt2[:, tsl]),
                "wf8": np.ascontiguousarray(wf8),
                "w1s": np.ascontiguousarray(w1t[hsl, :]),
            }
        )
    return in_maps


def run(x, w1, w2, w3, trace=False, trace_kwargs=None):
    from concourse.bass_utils import run_bass_kernel_spmd

    nc = _get_nc()
    in_maps = prepare_inputs(x, w1, w2, w3)
    res = run_bass_kernel_spmd(
        nc,
        in_maps,
        core_ids=list(range(NCORES)),
        trace=trace,
        **(trace_kwargs or {}),
    )
    out = np.empty((T, DIM), dtype=np.float64)
    for tg in range(NTG):
        tsl = slice(tg * TLOC, (tg + 1) * TLOC)
        accs = np.zeros((TLOC, DIM), dtype=np.float64)
        for hg in range(NHG):
            accs += res.results[tg * NHG + hg]["out"].astype(np.float64)
        out[tsl] = x.reshape(T, DIM)[tsl].astype(np.float64) + accs
    return out.astype(np.float32).reshape(x.shape), res


def kernel(x, w1, w2, w3):
    out, _ = run(np.asarray(x), np.asarray(w1), np.asarray(w2), np.asarray(w3))
    return out
